# revision 28
# baseline (speedup 1.0000x reference)
"""GQA kernel for Trainium2, 8 NeuronCores (DP over batch x TP over heads).

Problem (hardcoded): B=4, S=1024, EMBED=2048, HEADS=32, GROUPS=8,
GROUP_HEADS=4, HEAD_DIM=64.

Core c handles batch b = c//2 and TP half m = c%2 (16 heads = 4 groups).
All matmul operands are bf16 (PSUM accumulation stays fp32); host converts.

Device pipeline (single dense PE stream to keep the HAM clock gate at 8/8):
  [K proj][V proj][Q proj chunk 0]
  [16 attention slots: slot s = head pair p=s%8, token half = s//8.
     scores for the pair are row-tiled (partitions 0:64 / 64:128) so the
     two heads' 64-contract matmuls run concurrently; one 2048-wide exp
     per kc-pair covers both heads (4 PSUM banks); AV accumulates
     [dims|ones] so softmax denominators fall out of the matmul;
     1/den = Exp(-Ln(den)) on ACT (activation tables pinned so exp+ln
     share one table set); gpsimd partition broadcast; DVE muls.
     Fill work keeps the PE busy under the ACT-bound exp stream:
     slots 0-6 run Q-proj chunk p+1, slots 8-15 run the output projection
     for token half A (2 out-chunks per slot).]
  [FC tail: output projection for token half B]
Output is yT [E, S] bf16 (stationary-wfc FC); host transposes and reduces.
"""

import numpy as np
import ml_dtypes

import concourse.bass as bass
import concourse.tile as tile
from concourse import bacc, mybir
from concourse.bass_utils import run_bass_kernel_spmd
from concourse.hw_specs import get_activation_tables

F32 = mybir.dt.float32
BF16 = mybir.dt.bfloat16
AF = mybir.ActivationFunctionType

B, S, E = 4, 1024, 2048
HEADS_L = 16          # heads per core
GROUPS_L = 4          # groups per core
D = 64                # head dim
P = 128
NE = E // P           # 16 e-chunks
NT = S // P           # 8 token chunks
HO = HEADS_L * D      # 1024 local head-dims
GO = GROUPS_L * D     # 256 local group-dims
H2 = S // 2           # 512 = token half

_CACHE = {}


def _pin_act_tables(arch):
    """Keep Exp/Ln only in natural_log_exp_and_others so the table-load
    pass picks the one set covering every activation this kernel uses
    (1 ACT_TABLE_LOAD instead of one per Ln<->Exp alternation). Mutates
    the functools.cache'd dict in place; set indices are unchanged."""
    tabs = get_activation_tables(arch)
    for name, fns in tabs.items():
        if name != "natural_log_exp_and_others":
            fns.discard(AF.Exp)
            fns.discard(AF.Ln)


def _build():
    nc = bacc.Bacc("TRN2")
    _pin_act_tables(nc.m.arch)
    qT = nc.declare_dram_parameter("qT", [E, S], BF16, isOutput=False)
    kT = nc.declare_dram_parameter("kT", [E, S], BF16, isOutput=False)
    vT = nc.declare_dram_parameter("vT", [E, S], BF16, isOutput=False)
    wqT = nc.declare_dram_parameter("wqT", [E, HO], BF16, isOutput=False)
    wkT = nc.declare_dram_parameter("wkT", [E, GO], BF16, isOutput=False)
    wvT = nc.declare_dram_parameter("wvT", [E, GO], BF16, isOutput=False)
    wfcT = nc.declare_dram_parameter("wfcT", [HO, E], BF16, isOutput=False)
    y = nc.declare_dram_parameter("y", [E, S], BF16, isOutput=True)

    with tile.TileContext(nc) as tc:
        _body(nc, tc, qT, kT, vT, wqT, wkT, wvT, wfcT, y)
    nc.finalize()
    return nc


def _body(nc, tc, qT, kT, vT, wqT, wkT, wvT, wfcT, y):
    from contextlib import ExitStack
    with ExitStack() as ctx:
        # ---- persistent SBUF pools -----------------------------------
        p_kh = ctx.enter_context(tc.tile_pool(name="kh", bufs=GROUPS_L))
        p_vh = ctx.enter_context(tc.tile_pool(name="vh", bufs=NT))
        p_qh = ctx.enter_context(tc.tile_pool(name="qh", bufs=NT))
        p_ot = ctx.enter_context(tc.tile_pool(name="ot", bufs=NT))
        p_wfc = ctx.enter_context(tc.tile_pool(name="wfc", bufs=NT))
        p_avs = ctx.enter_context(tc.tile_pool(name="avs", bufs=2))
        p_rr = ctx.enter_context(tc.tile_pool(name="rr", bufs=1))
        p_r2 = ctx.enter_context(tc.tile_pool(name="r2", bufs=1))
        p_rb = ctx.enter_context(tc.tile_pool(name="rb", bufs=2))
        p_tmp = ctx.enter_context(tc.tile_pool(name="tmp", bufs=1))
        p_ysb = ctx.enter_context(tc.tile_pool(name="ysb", bufs=4))
        p_exp = ctx.enter_context(tc.tile_pool(name="exp", bufs=3))
        # stream pool reused for kte -> vte -> qte (qte lives to slot 6)
        p_in = ctx.enter_context(tc.tile_pool(name="instream", bufs=8))
        p_wq = ctx.enter_context(tc.tile_pool(name="wq", bufs=2))

        kh_dup = [p_kh.tile([P, S], BF16, tag="kh", name=f"khdup_{g}")
                  for g in range(GROUPS_L)]
        vh_aug = [p_vh.tile([P, GROUPS_L, D + 1], BF16, tag="vh", name=f"vhaug_{t}")
                  for t in range(NT)]
        qh_t = [p_qh.tile([P, S], BF16, tag="qh", name=f"qh_{t}") for t in range(NT)]
        ot_t = [p_ot.tile([P, S], BF16, tag="ot", name=f"ot_{t}") for t in range(NT)]
        wfc_t = [p_wfc.tile([P, E], BF16, tag="wfc", name=f"wfc_{i}") for i in range(NT)]
        _CACHE["tiles"] = {"kh": kh_dup, "vh": vh_aug, "qh": qh_t, "ot": ot_t}

        ones_col = nc.const_aps.tensor(1.0, (P, 1), BF16)
        ones4 = nc.const_aps.tensor(1.0, (P, GROUPS_L), BF16)
        for t in range(NT):
            nc.vector.tensor_copy(
                vh_aug[t][:, :, D:D + 1].rearrange("p g one -> p (g one)"),
                ones4)

        # e-major views of the transposed inputs/weights for batched DMA
        kT_r = kT.rearrange("(e p) t -> p e t", p=P)
        vT_r = vT.rearrange("(e p) t -> p e t", p=P)
        qT_r = qT.rearrange("(e p) t -> p e t", p=P)
        wkT_r = wkT.rearrange("(e p) c -> p e c", p=P)
        wvT_r = wvT.rearrange("(e p) c -> p e c", p=P)
        wqT_r = wqT.rearrange("(e p) c -> p e c", p=P)

        # PE warmup: ~5us of throwaway matmuls on const data so the HAM
        # clock gate reaches 8/8 while the first input DMAs land.
        warm_mv = nc.const_aps.tensor(1.0, (P, H2), BF16)
        with tc.tile_pool(name="pswarm", bufs=1, space="PSUM") as ps_w, \
             tc.tile_pool(name="sbwarm", bufs=1) as p_wsb:
            wps = ps_w.tile([P, H2], F32, tag="warm", name="warmps")
            wsb = p_wsb.tile([P, 1], F32, tag="wsb", name="warmsb")
            for w in range(24):
                nc.tensor.matmul(wps[0:1, :], ones_col, warm_mv,
                                 start=True, stop=True)
                if w == 0:
                    nc.scalar.activation(wsb[0:1, :], wps[0:1, 0:1], AF.Exp)

        # ---- phase 1: K/V projections --------------------------------
        with tc.tile_pool(name="wkv", bufs=1) as p_wkv:
            # interleave wk/kte chunk loads so K proj starts after ~1/4 of
            # the data; vte+wv right behind, qte after that; wfc is spread
            # across the attention slots (needed only from slot 8).
            wk_all = p_wkv.tile([P, NE, GO], BF16, tag="wk", name="wk_all")
            wv_all = p_wkv.tile([P, NE, GO], BF16, tag="wv", name="wv_all")
            kin, vin, qin = [], [], []
            for q4 in range(4):
                nc.sync.dma_start(out=wk_all[:, 4 * q4:4 * q4 + 4, :],
                                  in_=wkT_r[:, 4 * q4:4 * q4 + 4, :])
                t4 = p_in.tile([P, 4, S], BF16, tag="in", name=f"kte_{q4}")
                nc.sync.dma_start(out=t4, in_=kT_r[:, 4 * q4:4 * q4 + 4, :])
                kin.append(t4)
            for q4 in range(4):
                nc.sync.dma_start(out=wv_all[:, 4 * q4:4 * q4 + 4, :],
                                  in_=wvT_r[:, 4 * q4:4 * q4 + 4, :])
                t4 = p_in.tile([P, 4, S], BF16, tag="in", name=f"vte_{q4}")
                nc.sync.dma_start(out=t4, in_=vT_r[:, 4 * q4:4 * q4 + 4, :])
                vin.append(t4)
            for q4 in range(4):
                t4 = p_in.tile([P, 4, S], BF16, tag="in", name=f"qte_{q4}")
                nc.sync.dma_start(out=t4, in_=qT_r[:, 4 * q4:4 * q4 + 4, :])
                qin.append(t4)
            kte = lambda e, sl: kin[e // 4][:, e % 4, sl]
            vte = lambda e, sl: vin[e // 4][:, e % 4, sl]
            qte = lambda e, sl: qin[e // 4][:, e % 4, sl]

            # K projection: khT [256,1024]; o2-outer so drains overlap
            with tc.tile_pool(name="pk", bufs=2, space="PSUM") as ps_k:
                for o2 in range(2):
                    khps = ps_k.tile([P, S], F32, tag="pk", name=f"khps_{o2}")
                    for e in range(NE):
                        for t2 in range(2):
                            nc.tensor.matmul(
                                khps[:, t2 * H2:(t2 + 1) * H2],
                                wk_all[:, e, o2 * P:(o2 + 1) * P],
                                kte(e, slice(t2 * H2, (t2 + 1) * H2)),
                                start=(e == 0), stop=(e == NE - 1),
                            )
                    nc.scalar.activation(kh_dup[2 * o2][0:D, :],
                                         khps[0:D, :], AF.Copy)
                    nc.scalar.activation(kh_dup[2 * o2 + 1][D:P, :],
                                         khps[D:P, :], AF.Copy)
                for g in range(GROUPS_L):
                    if g % 2 == 0:
                        nc.gpsimd.dma_start(out=kh_dup[g][D:P, :],
                                            in_=kh_dup[g][0:D, :])
                    else:
                        nc.gpsimd.dma_start(out=kh_dup[g][0:D, :],
                                            in_=kh_dup[g][D:P, :])

            # V projection: vh [tokens, dims]; e-outer so compute starts
            # on the first vte DMA chunk; all 8 token-chunk accumulators
            # live in their own 8-bank pool.
            with tc.tile_pool(name="pv", bufs=8, space="PSUM") as ps_v:
                vps_t = [ps_v.tile([P, GO], F32, tag="pv", name=f"vps_{t}")
                         for t in range(NT)]
                for e in range(NE):
                    for t in range(NT):
                        nc.tensor.matmul(
                            vps_t[t][:, :],
                            vte(e, slice(t * P, (t + 1) * P)),
                            wv_all[:, e, :],
                            start=(e == 0), stop=(e == NE - 1),
                        )
                for t in range(NT):
                    nc.scalar.activation(
                        vh_aug[t][:, :, 0:D],
                        vps_t[t].rearrange("p (g d) -> p g d", g=GROUPS_L),
                        AF.Copy)


        def load_wq_chunk(o):
            w = p_wq.tile([P, NE, P], BF16, tag="wq", name=f"wq_{o}")
            nc.sync.dma_start(out=w, in_=wqT_r[:, :, o * P:(o + 1) * P])
            return w

        # ---- phase 2: Q proj + attention slots + FC ------------------
        # PSUM: sc 4 banks + av 2 banks + qy 2 banks = 8 banks
        from contextlib import ExitStack as _ES
        with tc.tile_pool(name="psqy", bufs=2, space="PSUM") as ps_qy:
            att_ctx = _ES()
            ps_sc = att_ctx.enter_context(
                tc.tile_pool(name="pssc", bufs=1, space="PSUM"))
            ps_av = att_ctx.enter_context(
                tc.tile_pool(name="psav", bufs=1, space="PSUM"))

            def q_proj_mms(wq_tile, qps2, kcp):
                # 16e x 2t2 = 32 matmuls spread over kcp groups of 8
                for e in range(4 * kcp, 4 * kcp + 4):
                    for t2 in range(2):
                        nc.tensor.matmul(
                            qps2[t2][:, :],
                            wq_tile[:, e, :],
                            qte(e, slice(t2 * H2, (t2 + 1) * H2)),
                            start=(e == 0), stop=(e == NE - 1),
                        )

            def drain_q(qps2, o):
                for t2 in range(2):
                    nc.vector.tensor_copy(
                        qh_t[o][:, t2 * H2:(t2 + 1) * H2], qps2[t2][:, :])

            def new_qps(o):
                return [ps_qy.tile([P, H2], F32, tag="pqy", name=f"qps_{o}_{t2}")
                        for t2 in range(2)]

            wq0 = load_wq_chunk(0)
            # Q chunk 0 up front (dense block, no fill needed yet)
            qps0 = new_qps(0)
            for kcp in range(4):
                q_proj_mms(wq0, qps0, kcp)
            drain_q(qps0, 0)
            wq_next = load_wq_chunk(1)

            def fc_chunk(eo, half, pool=None, tag="pqy"):
                tsl = slice(half * H2, (half + 1) * H2)
                yps = (pool or ps_qy).tile([P, H2], F32, tag=tag,
                                           name=f"yps_{half}_{eo}")
                for i in range(NT):
                    nc.tensor.matmul(
                        yps[:, :],
                        wfc_t[i][:, eo * P:(eo + 1) * P],
                        ot_t[i][:, tsl],
                        start=(i == 0), stop=(i == NT - 1),
                    )
                ysb = p_ysb.tile([P, H2], BF16, tag="ysb", name=f"ysb_{half}_{eo}")
                nc.vector.tensor_copy(ysb[:, :], yps[:, :])
                nc.sync.dma_start(out=y[eo * P:(eo + 1) * P, tsl], in_=ysb)

            # FC-A chunk schedule per half-B slot: slot 15 left empty so
            # the last pair's attention (and the FC-B tail behind it)
            # finishes as early as possible.
            fc_sched = {8: [0, 1], 9: [2, 3], 10: [4, 5], 11: [6, 7],
                        12: [8, 9], 13: [10, 11, 14], 14: [12, 13, 15],
                        15: []}
            for s in range(16):
                half, p = s // 8, s % 8
                g = p // 2
                tsl = slice(half * H2, (half + 1) * H2)
                if half == 0:
                    nc.sync.dma_start(out=wfc_t[p],
                                      in_=wfcT[p * P:(p + 1) * P, :])
                if half == 0 and p < 7:
                    wq_cur = wq_next
                    if p < 6:
                        wq_next = load_wq_chunk(p + 2)
                    qps = new_qps(p + 1)
                av = ps_av.tile([P, 2, H2], F32, tag="psav", name=f"av_{s}")
                exps = []
                for kcp in range(4):
                    sc = ps_sc.tile([P, 4, H2], F32, tag="pssc",
                                    name=f"sc_{s}_{kcp}")
                    for j in range(2):
                        kc = 2 * kcp + j
                        for i in range(2):
                            qb = i * D
                            nc.tensor.matmul(
                                sc[:, 2 * i + j, :],
                                kh_dup[g][qb:qb + D, kc * P:(kc + 1) * P],
                                qh_t[p][qb:qb + D, tsl],
                                start=True, stop=True,
                            )
                    ex = p_exp.tile([P, 4, H2], BF16, tag="exp",
                                    name=f"exp_{s}_{kcp}")
                    nc.scalar.activation(
                        ex.rearrange("p a b -> p (a b)"),
                        sc.rearrange("p a b -> p (a b)"), AF.Exp)
                    exps.append(ex)
                    # fill work for this kcp
                    if half == 0 and p < 7:
                        q_proj_mms(wq_cur, qps, kcp)
                    elif half == 1 and kcp < len(fc_sched[s]):
                        fc_chunk(fc_sched[s][kcp], 0)
                    # AV lagged two kcp stages: hides exp latency AND gives
                    # the previous slot's normalization chain time to free
                    # the (single-buffered) av banks before our first AV.
                    if kcp >= 2:
                        for j in range(2):
                            kc = 2 * (kcp - 2) + j
                            for i in range(2):
                                nc.tensor.matmul(
                                    av[0:D + 1, i, :],
                                    vh_aug[kc][:, g, :],
                                    exps[kcp - 2][:, 2 * i + j, :],
                                    start=(kc == 0), stop=False,
                                )
                for kcp in (2, 3):
                    for j in range(2):
                        kc = 2 * kcp + j
                        for i in range(2):
                            nc.tensor.matmul(
                                av[0:D + 1, i, :],
                                vh_aug[kc][:, g, :],
                                exps[kcp][:, 2 * i + j, :],
                                start=False, stop=(kc == NT - 1),
                            )
                if half == 0 and p < 7:
                    drain_q(qps, p + 1)
                # evacuate av to SBUF immediately so the single-buffered av
                # banks free ~1us after the AV tail instead of after the
                # whole normalization chain.
                avs = p_avs.tile([P, 2, H2], F32, tag="avs", name=f"avs_{s}")
                nc.vector.tensor_copy(
                    avs[0:D + 1, :, :].rearrange("p a b -> p (a b)"),
                    av[0:D + 1, :, :].rearrange("p a b -> p (a b)"))
                # normalization: 1/den = Exp(-Ln(den)) on ACT (both heads in
                # one 1024-wide pass), broadcast, then DVE muls.
                rr = p_rr.tile([P, 2, H2], F32, tag="rr", name=f"rr_{s}")
                r2 = p_r2.tile([P, 2, H2], F32, tag="r2", name=f"r2_{s}")
                rb = p_rb.tile([P, 2, H2], F32, tag="rb", name=f"rb_{s}")
                nc.scalar.activation(rr[D:D + 1, :, :].rearrange("p a b -> p (a b)"),
                                     avs[D:D + 1, :, :].rearrange("p a b -> p (a b)"),
                                     AF.Ln)
                nc.scalar.activation(r2[D:D + 1, :, :].rearrange("p a b -> p (a b)"),
                                     rr[D:D + 1, :, :].rearrange("p a b -> p (a b)"),
                                     AF.Exp, scale=-1.0)
                nc.gpsimd.dma_start(out=r2[0:1, :, :], in_=r2[D:D + 1, :, :])
                nc.gpsimd.partition_broadcast(
                    rb[0:D, :, :].rearrange("p a b -> p (a b)"),
                    r2[0:1, :, :].rearrange("p a b -> p (a b)"))
                tmp = p_tmp.tile([P, H2], BF16, tag="tmp", name=f"tmp_{s}")
                nc.vector.tensor_mul(tmp[0:D, :],
                                     avs[0:D, 1, :], rb[0:D, 1, :])
                nc.sync.dma_start(out=ot_t[p][D:P, tsl], in_=tmp[0:D, :])
                nc.vector.tensor_mul(ot_t[p][0:D, tsl],
                                     avs[0:D, 0, :], rb[0:D, 0, :])

            # FC tail: token half B — sc/av banks are released so six
            # FC chunks can be in flight. The first six chunks accumulate
            # i=0..6 before ANY i=7 matmul is issued: the in-order PE queue
            # then has ~9us of ready work hiding the last pair's
            # normalization latency.
            att_ctx.close()
            with tc.tile_pool(name="psyb", bufs=6, space="PSUM") as ps_yb:
                tslB = slice(H2, S)
                wave = []
                for eo in range(6):
                    yps = ps_yb.tile([P, H2], F32, tag="pyb",
                                     name=f"ypsB_{eo}")
                    for i in range(NT - 1):
                        nc.tensor.matmul(
                            yps[:, :], wfc_t[i][:, eo * P:(eo + 1) * P],
                            ot_t[i][:, tslB],
                            start=(i == 0), stop=False,
                        )
                    wave.append(yps)
                for eo in range(6):
                    yps = wave[eo]
                    nc.tensor.matmul(
                        yps[:, :], wfc_t[NT - 1][:, eo * P:(eo + 1) * P],
                        ot_t[NT - 1][:, tslB],
                        start=False, stop=True,
                    )
                    ysb = p_ysb.tile([P, H2], BF16, tag="ysb",
                                     name=f"ysbB_{eo}")
                    nc.vector.tensor_copy(ysb[:, :], yps[:, :])
                    nc.sync.dma_start(out=y[eo * P:(eo + 1) * P, tslB],
                                      in_=ysb)
                for eo in range(6, NE):
                    fc_chunk(eo, 1, pool=ps_yb, tag="pyb")


def _get_nc():
    if "nc" not in _CACHE:
        _CACHE["nc"] = _build()
    return _CACHE["nc"]


def _in_maps(q, k, v, Wq, Wk, Wv, Wfc):
    bf = ml_dtypes.bfloat16
    qTb = [np.ascontiguousarray(q[b].T).astype(bf) for b in range(B)]
    kTb = [np.ascontiguousarray(k[b].T).astype(bf) for b in range(B)]
    vTb = [np.ascontiguousarray(v[b].T).astype(bf) for b in range(B)]
    wqTm = [np.ascontiguousarray((Wq[m * HO:(m + 1) * HO, :] / 8.0).T).astype(bf)
            for m in range(2)]
    wkTm = [np.ascontiguousarray(Wk[m * GO:(m + 1) * GO, :].T).astype(bf)
            for m in range(2)]
    wvTm = [np.ascontiguousarray(Wv[m * GO:(m + 1) * GO, :].T).astype(bf)
            for m in range(2)]
    wfcTm = [np.ascontiguousarray(Wfc[:, m * HO:(m + 1) * HO].T).astype(bf)
             for m in range(2)]
    maps = []
    for c in range(8):
        b, m = c // 2, c % 2
        maps.append({
            "qT": qTb[b], "kT": kTb[b], "vT": vTb[b],
            "wqT": wqTm[m], "wkT": wkTm[m], "wvT": wvTm[m],
            "wfcT": wfcTm[m],
        })
    return maps


def kernel(q, k, v, Wq, Wk, Wv, Wfc, bfc):
    q = np.asarray(q, np.float32)
    k = np.asarray(k, np.float32)
    v = np.asarray(v, np.float32)
    Wq = np.asarray(Wq, np.float32)
    Wk = np.asarray(Wk, np.float32)
    Wv = np.asarray(Wv, np.float32)
    Wfc = np.asarray(Wfc, np.float32)
    bfc = np.asarray(bfc, np.float32)

    nc = _get_nc()
    res = run_bass_kernel_spmd(nc, _in_maps(q, k, v, Wq, Wk, Wv, Wfc),
                               list(range(8)))
    out = np.empty((B, S, E), np.float32)
    for b in range(B):
        yt = (res.results[2 * b]["y"].astype(np.float32)
              + res.results[2 * b + 1]["y"].astype(np.float32))
        out[b] = yt.T + bfc
    return out


# revision 29
# speedup vs baseline: 1.1578x; 1.1578x over previous
"""GQA kernel for Trainium2, 8 NeuronCores (DP over batch x TP over heads).

Problem (hardcoded): B=4, S=1024, EMBED=2048, HEADS=32, GROUPS=8,
GROUP_HEADS=4, HEAD_DIM=64.

Core c handles batch b = c//2 and TP half m = c%2 (16 heads = 4 groups).
All matmul operands are bf16 (PSUM accumulation stays fp32); host converts.

Device pipeline (single dense PE stream to keep the HAM clock gate at 8/8):
  [K proj][V proj][Q proj chunk 0]
  [16 attention slots: slot s = head pair p=s%8, token half = s//8.
     scores for the pair are row-tiled (partitions 0:64 / 64:128) so the
     two heads' 64-contract matmuls run concurrently; one 2048-wide exp
     per kc-pair covers both heads (4 PSUM banks); AV accumulates
     [dims|ones] so softmax denominators fall out of the matmul;
     1/den = Exp(-Ln(den)) on ACT (activation tables pinned so exp+ln
     share one table set); gpsimd partition broadcast; DVE muls.
     Fill work keeps the PE busy under the ACT-bound exp stream:
     slots 0-6 run Q-proj chunk p+1, slots 8-15 run the output projection
     for token half A (2 out-chunks per slot).]
  [FC tail: output projection for token half B]
Output is yT [E, S] bf16 (stationary-wfc FC); host transposes and reduces.
"""

import numpy as np
import ml_dtypes

import concourse.bass as bass
import concourse.tile as tile
from concourse import bacc, mybir
from concourse.bass_utils import run_bass_kernel_spmd
from concourse.hw_specs import get_activation_tables

F32 = mybir.dt.float32
BF16 = mybir.dt.bfloat16
AF = mybir.ActivationFunctionType

B, S, E = 4, 1024, 2048
HEADS_L = 16          # heads per core
GROUPS_L = 4          # groups per core
D = 64                # head dim
P = 128
NE = E // P           # 16 e-chunks
NT = S // P           # 8 token chunks
HO = HEADS_L * D      # 1024 local head-dims
GO = GROUPS_L * D     # 256 local group-dims
H2 = S // 2           # 512 = token half

_CACHE = {}


def _pin_act_tables(arch):
    """Keep Exp/Ln only in natural_log_exp_and_others so the table-load
    pass picks the one set covering every activation this kernel uses
    (1 ACT_TABLE_LOAD instead of one per Ln<->Exp alternation). Mutates
    the functools.cache'd dict in place; set indices are unchanged."""
    tabs = get_activation_tables(arch)
    for name, fns in tabs.items():
        if name != "natural_log_exp_and_others":
            fns.discard(AF.Exp)
            fns.discard(AF.Ln)


def _build():
    nc = bacc.Bacc("TRN2")
    _pin_act_tables(nc.m.arch)
    qT = nc.declare_dram_parameter("qT", [E, S], BF16, isOutput=False)
    kT = nc.declare_dram_parameter("kT", [E, S], BF16, isOutput=False)
    vT = nc.declare_dram_parameter("vT", [E, S], BF16, isOutput=False)
    wqT = nc.declare_dram_parameter("wqT", [E, HO], BF16, isOutput=False)
    wkT = nc.declare_dram_parameter("wkT", [E, GO], BF16, isOutput=False)
    wvT = nc.declare_dram_parameter("wvT", [E, GO], BF16, isOutput=False)
    wfcT = nc.declare_dram_parameter("wfcT", [HO, E], BF16, isOutput=False)
    y = nc.declare_dram_parameter("y", [E, S], BF16, isOutput=True)

    with tile.TileContext(nc) as tc:
        _body(nc, tc, qT, kT, vT, wqT, wkT, wvT, wfcT, y)
    nc.finalize()
    return nc


def _body(nc, tc, qT, kT, vT, wqT, wkT, wvT, wfcT, y):
    from contextlib import ExitStack
    with ExitStack() as ctx:
        # ---- persistent SBUF pools -----------------------------------
        p_kh = ctx.enter_context(tc.tile_pool(name="kh", bufs=GROUPS_L))
        p_vh = ctx.enter_context(tc.tile_pool(name="vh", bufs=NT))
        p_qh = ctx.enter_context(tc.tile_pool(name="qh", bufs=NT))
        p_ot = ctx.enter_context(tc.tile_pool(name="ot", bufs=NT))
        p_wfc = ctx.enter_context(tc.tile_pool(name="wfc", bufs=NT))
        p_avs = ctx.enter_context(tc.tile_pool(name="avs", bufs=2))
        p_rr = ctx.enter_context(tc.tile_pool(name="rr", bufs=1))
        p_r2 = ctx.enter_context(tc.tile_pool(name="r2", bufs=1))
        p_rb = ctx.enter_context(tc.tile_pool(name="rb", bufs=2))
        p_tmp = ctx.enter_context(tc.tile_pool(name="tmp", bufs=1))
        p_ysb = ctx.enter_context(tc.tile_pool(name="ysb", bufs=4))
        p_exp = ctx.enter_context(tc.tile_pool(name="exp", bufs=3))
        # stream pool reused for kte -> vte -> qte (qte lives to slot 6)
        p_in = ctx.enter_context(tc.tile_pool(name="instream", bufs=8))
        p_wq = ctx.enter_context(tc.tile_pool(name="wq", bufs=2))

        kh_dup = [p_kh.tile([P, S], BF16, tag="kh", name=f"khdup_{g}")
                  for g in range(GROUPS_L)]
        vh_aug = [p_vh.tile([P, GROUPS_L, D + 1], BF16, tag="vh", name=f"vhaug_{t}")
                  for t in range(NT)]
        qh_t = [p_qh.tile([P, S], BF16, tag="qh", name=f"qh_{t}") for t in range(NT)]
        ot_t = [p_ot.tile([P, S], BF16, tag="ot", name=f"ot_{t}") for t in range(NT)]
        wfc_t = [p_wfc.tile([P, E], BF16, tag="wfc", name=f"wfc_{i}") for i in range(NT)]
        _CACHE["tiles"] = {"kh": kh_dup, "vh": vh_aug, "qh": qh_t, "ot": ot_t}

        ones_col = nc.const_aps.tensor(1.0, (P, 1), BF16)
        ones4 = nc.const_aps.tensor(1.0, (P, GROUPS_L), BF16)
        for t in range(NT):
            nc.vector.tensor_copy(
                vh_aug[t][:, :, D:D + 1].rearrange("p g one -> p (g one)"),
                ones4)

        # e-major views of the transposed inputs/weights for batched DMA
        kT_r = kT.rearrange("(e p) t -> p e t", p=P)
        vT_r = vT.rearrange("(e p) t -> p e t", p=P)
        qT_r = qT.rearrange("(e p) t -> p e t", p=P)
        wkT_r = wkT.rearrange("(e p) c -> p e c", p=P)
        wvT_r = wvT.rearrange("(e p) c -> p e c", p=P)
        wqT_r = wqT.rearrange("(e p) c -> p e c", p=P)

        # PE warmup: ~5us of throwaway matmuls on const data so the HAM
        # clock gate reaches 8/8 while the first input DMAs land.
        warm_mv = nc.const_aps.tensor(1.0, (P, H2), BF16)
        with tc.tile_pool(name="pswarm", bufs=1, space="PSUM") as ps_w, \
             tc.tile_pool(name="sbwarm", bufs=1) as p_wsb:
            wps = ps_w.tile([P, H2], F32, tag="warm", name="warmps")
            wsb = p_wsb.tile([P, 1], F32, tag="wsb", name="warmsb")
            for w in range(24):
                nc.tensor.matmul(wps[0:1, :], ones_col, warm_mv,
                                 start=True, stop=True)
                if w == 0:
                    nc.scalar.activation(wsb[0:1, :], wps[0:1, 0:1], AF.Exp, scale=-1.0)

        # ---- phase 1: K/V projections --------------------------------
        with tc.tile_pool(name="wkv", bufs=1) as p_wkv:
            # interleave wk/kte chunk loads so K proj starts after ~1/4 of
            # the data; vte+wv right behind, qte after that; wfc is spread
            # across the attention slots (needed only from slot 8).
            wk_all = p_wkv.tile([P, NE, GO], BF16, tag="wk", name="wk_all")
            wv_all = p_wkv.tile([P, NE, GO], BF16, tag="wv", name="wv_all")
            kin, vin, qin = [], [], []
            for q4 in range(4):
                nc.sync.dma_start(out=wk_all[:, 4 * q4:4 * q4 + 4, :],
                                  in_=wkT_r[:, 4 * q4:4 * q4 + 4, :])
                t4 = p_in.tile([P, 4, S], BF16, tag="in", name=f"kte_{q4}")
                nc.sync.dma_start(out=t4, in_=kT_r[:, 4 * q4:4 * q4 + 4, :])
                kin.append(t4)
            for q4 in range(4):
                nc.sync.dma_start(out=wv_all[:, 4 * q4:4 * q4 + 4, :],
                                  in_=wvT_r[:, 4 * q4:4 * q4 + 4, :])
                t4 = p_in.tile([P, 4, S], BF16, tag="in", name=f"vte_{q4}")
                nc.sync.dma_start(out=t4, in_=vT_r[:, 4 * q4:4 * q4 + 4, :])
                vin.append(t4)
            for q4 in range(4):
                t4 = p_in.tile([P, 4, S], BF16, tag="in", name=f"qte_{q4}")
                nc.sync.dma_start(out=t4, in_=qT_r[:, 4 * q4:4 * q4 + 4, :])
                qin.append(t4)
            kte = lambda e, sl: kin[e // 4][:, e % 4, sl]
            vte = lambda e, sl: vin[e // 4][:, e % 4, sl]
            qte = lambda e, sl: qin[e // 4][:, e % 4, sl]

            # K projection: khT [256,1024]; o2-outer so drains overlap
            with tc.tile_pool(name="pk", bufs=2, space="PSUM") as ps_k:
                for o2 in range(2):
                    khps = ps_k.tile([P, S], F32, tag="pk", name=f"khps_{o2}")
                    for e in range(NE):
                        for t2 in range(2):
                            nc.tensor.matmul(
                                khps[:, t2 * H2:(t2 + 1) * H2],
                                wk_all[:, e, o2 * P:(o2 + 1) * P],
                                kte(e, slice(t2 * H2, (t2 + 1) * H2)),
                                start=(e == 0), stop=(e == NE - 1),
                            )
                    nc.scalar.activation(kh_dup[2 * o2][0:D, :],
                                         khps[0:D, :], AF.Copy)
                    nc.scalar.activation(kh_dup[2 * o2 + 1][D:P, :],
                                         khps[D:P, :], AF.Copy)
                for g in range(GROUPS_L):
                    if g % 2 == 0:
                        nc.gpsimd.dma_start(out=kh_dup[g][D:P, :],
                                            in_=kh_dup[g][0:D, :])
                    else:
                        nc.gpsimd.dma_start(out=kh_dup[g][0:D, :],
                                            in_=kh_dup[g][D:P, :])

            # V projection: vh [tokens, dims]; e-outer so compute starts
            # on the first vte DMA chunk; all 8 token-chunk accumulators
            # live in their own 8-bank pool.
            with tc.tile_pool(name="pv", bufs=8, space="PSUM") as ps_v:
                vps_t = [ps_v.tile([P, GO], F32, tag="pv", name=f"vps_{t}")
                         for t in range(NT)]
                for e in range(NE):
                    for t in range(NT):
                        nc.tensor.matmul(
                            vps_t[t][:, :],
                            vte(e, slice(t * P, (t + 1) * P)),
                            wv_all[:, e, :],
                            start=(e == 0), stop=(e == NE - 1),
                        )
                for t in range(NT):
                    nc.scalar.activation(
                        vh_aug[t][:, :, 0:D],
                        vps_t[t].rearrange("p (g d) -> p g d", g=GROUPS_L),
                        AF.Copy)


        def load_wq_chunk(o):
            w = p_wq.tile([P, NE, P], BF16, tag="wq", name=f"wq_{o}")
            nc.sync.dma_start(out=w, in_=wqT_r[:, :, o * P:(o + 1) * P])
            return w

        # ---- phase 2: Q proj + attention slots + FC ------------------
        # PSUM: sc 4 banks + av 2 banks + qy 2 banks = 8 banks
        from contextlib import ExitStack as _ES
        with tc.tile_pool(name="psqy", bufs=2, space="PSUM") as ps_qy:
            att_ctx = _ES()
            ps_sc = att_ctx.enter_context(
                tc.tile_pool(name="pssc", bufs=1, space="PSUM"))
            ps_av = att_ctx.enter_context(
                tc.tile_pool(name="psav", bufs=1, space="PSUM"))

            def q_proj_mms(wq_tile, qps2, kcp):
                # 16e x 2t2 = 32 matmuls spread over kcp groups of 8
                for e in range(4 * kcp, 4 * kcp + 4):
                    for t2 in range(2):
                        nc.tensor.matmul(
                            qps2[t2][:, :],
                            wq_tile[:, e, :],
                            qte(e, slice(t2 * H2, (t2 + 1) * H2)),
                            start=(e == 0), stop=(e == NE - 1),
                        )

            def drain_q(qps2, o):
                for t2 in range(2):
                    nc.vector.tensor_copy(
                        qh_t[o][:, t2 * H2:(t2 + 1) * H2], qps2[t2][:, :])

            def new_qps(o):
                return [ps_qy.tile([P, H2], F32, tag="pqy", name=f"qps_{o}_{t2}")
                        for t2 in range(2)]

            wq0 = load_wq_chunk(0)
            # Q chunk 0 up front (dense block, no fill needed yet)
            qps0 = new_qps(0)
            for kcp in range(4):
                q_proj_mms(wq0, qps0, kcp)
            drain_q(qps0, 0)
            wq_next = load_wq_chunk(1)

            def fc_chunk(eo, half, pool=None, tag="pqy"):
                tsl = slice(half * H2, (half + 1) * H2)
                yps = (pool or ps_qy).tile([P, H2], F32, tag=tag,
                                           name=f"yps_{half}_{eo}")
                for i in range(NT):
                    nc.tensor.matmul(
                        yps[:, :],
                        wfc_t[i][:, eo * P:(eo + 1) * P],
                        ot_t[i][:, tsl],
                        start=(i == 0), stop=(i == NT - 1),
                    )
                ysb = p_ysb.tile([P, H2], BF16, tag="ysb", name=f"ysb_{half}_{eo}")
                nc.vector.tensor_copy(ysb[:, :], yps[:, :])
                nc.sync.dma_start(out=y[eo * P:(eo + 1) * P, tsl], in_=ysb)

            # FC-A chunk schedule per half-B slot: slot 15 left empty so
            # the last pair's attention (and the FC-B tail behind it)
            # finishes as early as possible.
            fc_sched = {8: [0, 1], 9: [2, 3], 10: [4, 5], 11: [6, 7],
                        12: [8, 9], 13: [10, 11, 14], 14: [12, 13, 15],
                        15: []}
            for s in range(16):
                half, p = s // 8, s % 8
                g = p // 2
                tsl = slice(half * H2, (half + 1) * H2)
                if half == 0:
                    nc.sync.dma_start(out=wfc_t[p],
                                      in_=wfcT[p * P:(p + 1) * P, :])
                if half == 0 and p < 7:
                    wq_cur = wq_next
                    if p < 6:
                        wq_next = load_wq_chunk(p + 2)
                    qps = new_qps(p + 1)
                av = ps_av.tile([P, 2, H2], F32, tag="psav", name=f"av_{s}")
                exps = []
                for kcp in range(4):
                    sc = ps_sc.tile([P, 4, H2], F32, tag="pssc",
                                    name=f"sc_{s}_{kcp}")
                    for j in range(2):
                        kc = 2 * kcp + j
                        for i in range(2):
                            qb = i * D
                            nc.tensor.matmul(
                                sc[:, 2 * i + j, :],
                                kh_dup[g][qb:qb + D, kc * P:(kc + 1) * P],
                                qh_t[p][qb:qb + D, tsl],
                                start=True, stop=True,
                            )
                    ex = p_exp.tile([P, 4, H2], BF16, tag="exp",
                                    name=f"exp_{s}_{kcp}")
                    nc.scalar.activation(
                        ex.rearrange("p a b -> p (a b)"),
                        sc.rearrange("p a b -> p (a b)"), AF.Exp)
                    exps.append(ex)
                    # fill work for this kcp
                    if half == 0 and p < 7:
                        q_proj_mms(wq_cur, qps, kcp)
                    elif half == 1 and kcp < len(fc_sched[s]):
                        fc_chunk(fc_sched[s][kcp], 0)
                    # AV lagged two kcp stages: hides exp latency AND gives
                    # the previous slot's normalization chain time to free
                    # the (single-buffered) av banks before our first AV.
                    if kcp >= 2:
                        for j in range(2):
                            kc = 2 * (kcp - 2) + j
                            for i in range(2):
                                nc.tensor.matmul(
                                    av[0:D + 1, i, :],
                                    vh_aug[kc][:, g, :],
                                    exps[kcp - 2][:, 2 * i + j, :],
                                    start=(kc == 0), stop=False,
                                )
                for kcp in (2, 3):
                    for j in range(2):
                        kc = 2 * kcp + j
                        for i in range(2):
                            nc.tensor.matmul(
                                av[0:D + 1, i, :],
                                vh_aug[kc][:, g, :],
                                exps[kcp][:, 2 * i + j, :],
                                start=False, stop=(kc == NT - 1),
                            )
                if half == 0 and p < 7:
                    drain_q(qps, p + 1)
                # evacuate av to SBUF immediately so the single-buffered av
                # banks free ~1us after the AV tail instead of after the
                # whole normalization chain.
                avs = p_avs.tile([P, 2, H2], F32, tag="avs", name=f"avs_{s}")
                nc.vector.tensor_copy(
                    avs[0:D + 1, :, :].rearrange("p a b -> p (a b)"),
                    av[0:D + 1, :, :].rearrange("p a b -> p (a b)"))
                # normalization: 1/den = Exp(-Ln(den)) on ACT (both heads in
                # one 1024-wide pass), broadcast, then DVE muls.
                rr = p_rr.tile([P, 2, H2], F32, tag="rr", name=f"rr_{s}")
                r2 = p_r2.tile([P, 2, H2], F32, tag="r2", name=f"r2_{s}")
                rb = p_rb.tile([P, 2, H2], F32, tag="rb", name=f"rb_{s}")
                nc.scalar.activation(rr[D:D + 1, :, :].rearrange("p a b -> p (a b)"),
                                     avs[D:D + 1, :, :].rearrange("p a b -> p (a b)"),
                                     AF.Ln)
                nc.scalar.activation(r2[D:D + 1, :, :].rearrange("p a b -> p (a b)"),
                                     rr[D:D + 1, :, :].rearrange("p a b -> p (a b)"),
                                     AF.Exp, scale=-1.0)
                nc.gpsimd.dma_start(out=r2[0:1, :, :], in_=r2[D:D + 1, :, :])
                nc.gpsimd.partition_broadcast(
                    rb[0:D, :, :].rearrange("p a b -> p (a b)"),
                    r2[0:1, :, :].rearrange("p a b -> p (a b)"))
                tmp = p_tmp.tile([P, H2], BF16, tag="tmp", name=f"tmp_{s}")
                nc.vector.tensor_mul(tmp[0:D, :],
                                     avs[0:D, 1, :], rb[0:D, 1, :])
                nc.sync.dma_start(out=ot_t[p][D:P, tsl], in_=tmp[0:D, :])
                nc.vector.tensor_mul(ot_t[p][0:D, tsl],
                                     avs[0:D, 0, :], rb[0:D, 0, :])

            # FC tail: token half B — sc/av banks are released so six
            # FC chunks can be in flight. The first six chunks accumulate
            # i=0..6 before ANY i=7 matmul is issued: the in-order PE queue
            # then has ~9us of ready work hiding the last pair's
            # normalization latency.
            att_ctx.close()
            with tc.tile_pool(name="psyb", bufs=6, space="PSUM") as ps_yb:
                tslB = slice(H2, S)
                wave = []
                for eo in range(6):
                    yps = ps_yb.tile([P, H2], F32, tag="pyb",
                                     name=f"ypsB_{eo}")
                    for i in range(NT - 1):
                        nc.tensor.matmul(
                            yps[:, :], wfc_t[i][:, eo * P:(eo + 1) * P],
                            ot_t[i][:, tslB],
                            start=(i == 0), stop=False,
                        )
                    wave.append(yps)
                for eo in range(6):
                    yps = wave[eo]
                    nc.tensor.matmul(
                        yps[:, :], wfc_t[NT - 1][:, eo * P:(eo + 1) * P],
                        ot_t[NT - 1][:, tslB],
                        start=False, stop=True,
                    )
                    ysb = p_ysb.tile([P, H2], BF16, tag="ysb",
                                     name=f"ysbB_{eo}")
                    nc.vector.tensor_copy(ysb[:, :], yps[:, :])
                    nc.sync.dma_start(out=y[eo * P:(eo + 1) * P, tslB],
                                      in_=ysb)
                for eo in range(6, NE):
                    fc_chunk(eo, 1, pool=ps_yb, tag="pyb")


def _get_nc():
    if "nc" not in _CACHE:
        _CACHE["nc"] = _build()
    return _CACHE["nc"]


def _in_maps(q, k, v, Wq, Wk, Wv, Wfc):
    bf = ml_dtypes.bfloat16
    qTb = [np.ascontiguousarray(q[b].T).astype(bf) for b in range(B)]
    kTb = [np.ascontiguousarray(k[b].T).astype(bf) for b in range(B)]
    vTb = [np.ascontiguousarray(v[b].T).astype(bf) for b in range(B)]
    wqTm = [np.ascontiguousarray((Wq[m * HO:(m + 1) * HO, :] / 8.0).T).astype(bf)
            for m in range(2)]
    wkTm = [np.ascontiguousarray(Wk[m * GO:(m + 1) * GO, :].T).astype(bf)
            for m in range(2)]
    wvTm = [np.ascontiguousarray(Wv[m * GO:(m + 1) * GO, :].T).astype(bf)
            for m in range(2)]
    wfcTm = [np.ascontiguousarray(Wfc[:, m * HO:(m + 1) * HO].T).astype(bf)
             for m in range(2)]
    maps = []
    for c in range(8):
        b, m = c // 2, c % 2
        maps.append({
            "qT": qTb[b], "kT": kTb[b], "vT": vTb[b],
            "wqT": wqTm[m], "wkT": wkTm[m], "wvT": wvTm[m],
            "wfcT": wfcTm[m],
        })
    return maps


def kernel(q, k, v, Wq, Wk, Wv, Wfc, bfc):
    q = np.asarray(q, np.float32)
    k = np.asarray(k, np.float32)
    v = np.asarray(v, np.float32)
    Wq = np.asarray(Wq, np.float32)
    Wk = np.asarray(Wk, np.float32)
    Wv = np.asarray(Wv, np.float32)
    Wfc = np.asarray(Wfc, np.float32)
    bfc = np.asarray(bfc, np.float32)

    nc = _get_nc()
    res = run_bass_kernel_spmd(nc, _in_maps(q, k, v, Wq, Wk, Wv, Wfc),
                               list(range(8)))
    out = np.empty((B, S, E), np.float32)
    for b in range(B):
        yt = (res.results[2 * b]["y"].astype(np.float32)
              + res.results[2 * b + 1]["y"].astype(np.float32))
        out[b] = yt.T + bfc
    return out
